# revision 17
# baseline (speedup 1.0000x reference)
"""8-core SPMD multi-head attention kernel for Trainium2 (Bass/Tile).

Problem: nn.MultiHeadAttention, B=2, S=2048, d_model=1024, 16 heads (dk=64).

Sharding: tensor-parallel over heads — 2 heads per core. Q/K/V projection
weights are column-split per core, out-projection row-split; each core
produces a partial [1024, 4096] output that the host sums.

v3 structure: one monolithic attention stream over (batch, 512-q-block)
with filler work interleaved between chunk-pairs so the PE never idles
long (long PE gaps were measured to reset the clock boost and cost ~35us):
  - proj(b0) + vh(b0) up front with j-interleaved streamed DMA;
  - attention(b0) with proj(b1)+vh(b1) interleaved (PSUM: attention needs
    only 6 banks in the [128, 2, 512] chunk-paired layout, leaving 2);
  - attention(b1) with outproj(b0) interleaved;
  - outproj(b1) tail.
Scores psum is [128, 2, 512] (chunk-paired): exp stays 1024-wide (ACT
cost unchanged) and the pair axis is exactly the fp8 DoubleRow k-subtile
layout, so AV runs e4m3 DoubleRow at half cost. Out-projection in bf16.
"""

import sys

sys.path.insert(0, "/opt/trn_rl_repo")

import numpy as np
import ml_dtypes

import concourse.bass as bass  # noqa: F401
import concourse.mybir as mybir
import concourse.tile as tile
from concourse import bacc
from concourse import bass_utils
from concourse.masks import make_identity

B, S, DM, H, DK = 2, 2048, 1024, 16, 64
TOK = B * S
NCORES = 8
HPC = H // NCORES    # 2 heads per core
CW = HPC * DK        # 128 = per-core qkv width
KC = DM // 128       # 8 contraction chunks
KCH = S // 128       # 16 k-token chunks per batch
NCP = KCH // 2       # 8 chunk-pairs
NQB = S // 512       # 4 q-blocks per batch
F32 = mybir.dt.float32
BF16 = mybir.dt.bfloat16
FP8 = mybir.dt.float8e4
AFT = mybir.ActivationFunctionType
DR = mybir.MatmulPerfMode.DoubleRow

TRACE = False
LAST_EXEC_NS = None

_compiled = None
_ONES = np.ones((128, 64), np.float32).astype(ml_dtypes.bfloat16)


def _build():
    nc = bacc.Bacc("TRN2", target_bir_lowering=False, debug=False,
                   num_devices=NCORES)

    qT = nc.dram_tensor("qT", [DM, TOK], BF16, kind="ExternalInput").ap()
    kT = nc.dram_tensor("kT", [DM, TOK], BF16, kind="ExternalInput").ap()
    vT = nc.dram_tensor("vT", [DM, TOK], BF16, kind="ExternalInput").ap()
    wq = nc.dram_tensor("wq", [128, KC * CW], BF16, kind="ExternalInput").ap()
    wk = nc.dram_tensor("wk", [128, KC * CW], BF16, kind="ExternalInput").ap()
    wv = nc.dram_tensor("wv", [128, KC * CW], BF16, kind="ExternalInput").ap()
    bq = nc.dram_tensor("bq", [128, 1], F32, kind="ExternalInput").ap()
    bk = nc.dram_tensor("bk", [128, 1], F32, kind="ExternalInput").ap()
    bv = nc.dram_tensor("bv", [128, 1], F32, kind="ExternalInput").ap()
    wo = nc.dram_tensor("wo", [CW, DM], BF16, kind="ExternalInput").ap()
    bo8 = nc.dram_tensor("bo8", [128, 8], F32, kind="ExternalInput").ap()
    onesd = nc.dram_tensor("onesd", [128, 64], BF16, kind="ExternalInput").ap()
    out = nc.dram_tensor("out", [DM, TOK], BF16, kind="ExternalOutput").ap()

    def xview(x):
        # [p, c, t]: partition p of chunk c holds feature c*128+p
        return x.rearrange("(c p) t -> p c t", c=KC)

    with tile.TileContext(nc) as tc, \
         tc.tile_pool(name="const", bufs=1) as const, \
         tc.tile_pool(name="xin", bufs=48) as xin, \
         tc.tile_pool(name="expp", bufs=2) as expp, \
         tc.tile_pool(name="stage", bufs=2) as stage, \
         tc.tile_pool(name="outst", bufs=4) as outst:

        # ---------- constants & persistent buffers ----------
        wq_sb = const.tile([128, KC, CW], BF16, tag="wq")
        wk_sb = const.tile([128, KC, CW], BF16, tag="wk")
        wv_sb = const.tile([128, KC, CW], BF16, tag="wv")
        bq_sb = const.tile([128, 1], F32, tag="bq")
        bk_sb = const.tile([128, 1], F32, tag="bk")
        bv_sb = const.tile([128, 1], F32, tag="bv")

        def wpieces(sb, dram):
            dv = dram.rearrange("p (c m) -> p c m", c=KC)
            for i in range(4):
                nc.sync.dma_start(sb[:, 2 * i:2 * i + 2, :],
                                  dv[:, 2 * i:2 * i + 2, :])

        # k weights first: the first projection group only waits on these
        wpieces(wk_sb, wk)
        nc.sync.dma_start(bk_sb[:], bk[:])
        wpieces(wq_sb, wq)
        nc.sync.dma_start(bq_sb[:], bq[:])
        wo_sb = const.tile([CW, DM], BF16, tag="wo")
        bo_sb = const.tile([128, 8], F32, tag="bo")
        ident = const.tile([128, 128], BF16, tag="ident")
        ones_sb = const.tile([128, 64], BF16, tag="ones_sb")

        qhT = [const.tile([128, S], BF16, tag=f"qhT{b}", name=f"qhT{b}")
               for b in range(B)]
        khT = [const.tile([128, S], BF16, tag=f"khT{b}", name=f"khT{b}")
               for b in range(B)]
        vhT = [const.tile([128, S], BF16, tag=f"vhT{b}", name=f"vhT{b}")
               for b in range(B)]
        ctxT = [const.tile([128, S], BF16, tag=f"ctxT{b}", name=f"ctxT{b}")
                for b in range(B)]
        vh = const.tile([128, B, HPC, KCH, DK + 1], BF16, tag="vh")

        PROJ = ((kT, wk_sb, bk_sb, khT), (qT, wq_sb, bq_sb, qhT),
                (vT, wv_sb, bv_sb, vhT))

        def late_consts():
            wpieces(wv_sb, wv)
            nc.sync.dma_start(bv_sb[:], bv[:])
            nc.sync.dma_start(wo_sb[:, 0:512], wo[:, 0:512])
            nc.sync.dma_start(wo_sb[:, 512:1024], wo[:, 512:1024])
            nc.sync.dma_start(bo_sb[:], bo8[:])
            make_identity(nc, ident[:])
            nc.sync.dma_start(ones_sb[:], onesd[:])
            nc.vector.tensor_copy(
                vh[:, :, :, :, DK:DK + 1],
                ones_sb[:].rearrange("p (a b c d) -> p a b c d",
                                     a=B, b=HPC, c=KCH, d=1))

        def issue_input_dma(b, order, fine=()):
            # j-round-interleaved so early chunks of k/q/v all land early;
            # groups in `fine` split per-chunk so transfers fan across queues
            tiles = {}
            for ti, j in order:
                xv = xview(PROJ[ti][0])
                t0 = b * S + j * 512
                for cp in range(4):
                    t = xin.tile([128, 2, 512], BF16, tag="xt")
                    if (ti, j) in fine:
                        for i in range(2):
                            nc.sync.dma_start(
                                t[:, i, :],
                                xv[:, 2 * cp + i, t0:t0 + 512])
                    else:
                        nc.sync.dma_start(
                            t[:], xv[:, 2 * cp:2 * cp + 2, t0:t0 + 512])
                    tiles[(ti, j, cp)] = t
            return tiles

        def proj_group(b, tiles, pp, ti, j, evict_act=False):
            _, w_sb, b_sb, dst = PROJ[ti]
            ps = pp.tile([128, 512], F32, tag="pp", name=f"pp{b}{ti}{j}")
            for c in range(KC):
                nc.tensor.matmul(
                    ps[:], w_sb[:, c, :],
                    tiles[(ti, j, c // 2)][:, c % 2, :],
                    start=(c == 0), stop=(c == KC - 1))
            d = dst[b][:, j * 512:(j + 1) * 512]
            if evict_act:
                nc.scalar.activation(d, ps[:], AFT.Identity, bias=b_sb[:])
            else:
                nc.vector.tensor_scalar_add(d, ps[:], b_sb[:])

        def vh_unit(b, ptr, h, c, evict_act=False):
            pt = ptr.tile([128, 64], BF16, tag="pt")
            hb = h * 64
            nc.tensor.transpose(
                pt[:], vhT[b][hb:hb + 64, c * 128:(c + 1) * 128],
                ident[hb:hb + 64, hb:hb + 64])
            if evict_act:
                nc.scalar.activation(vh[:, b, h, c, 0:DK], pt[:], AFT.Copy)
            else:
                nc.vector.tensor_copy(vh[:, b, h, c, 0:DK], pt[:])

        def outproj_unit(b, pout, od, qb, evict_act=False):
            po = pout.tile([128, 512], F32, tag="po")
            nc.tensor.matmul(
                po[:], wo_sb[:, od * 128:(od + 1) * 128],
                ctxT[b][:, qb * 512:(qb + 1) * 512],
                start=True, stop=True)
            ot = outst.tile([128, 512], BF16, tag="ot")
            if evict_act:
                nc.scalar.activation(ot[:], po[:], AFT.Identity,
                                     bias=bo_sb[:, od:od + 1])
            else:
                nc.vector.tensor_scalar_add(ot[:], po[:], bo_sb[:, od:od + 1])
            nc.sync.dma_start(
                out[od * 128:(od + 1) * 128,
                    b * S + qb * 512:b * S + (qb + 1) * 512],
                ot[:])

        def attention_batch(b, patt, filler, front=0):
            """Attention for batch b over 4 q-blocks of 512, draining
            `filler` between chunk-pairs; the first `front` units drain at
            2/cp (data-streaming deadlines), the rest evenly."""
            fi = 0

            def quota(slot):
                # slot = global cp index 0..31
                got = min(2 * (slot + 1), front)
                rest = len(filler) - front
                if rest > 0:
                    got += (slot + 1) * rest // 32
                return min(got, len(filler))
            for qb in range(NQB):
                q0 = qb * 512
                pctx = [patt.tile([DK + 1, 512], F32, tag=f"pctx{h}",
                                  name=f"pctx{b}_{qb}_{h}")
                        for h in range(HPC)]

                def do_av(cp, ets):
                    for h in range(HPC):
                        for i in range(2):
                            nc.tensor.matmul(
                                pctx[h][:], vh[:, b, h, 2 * cp + i, :],
                                ets[h][:, i, :],
                                start=(cp == 0 and i == 0),
                                stop=(cp == NCP - 1 and i == 1))

                pend = None
                for cp in range(NCP):
                    pss = [patt.tile([128, 2, 512], F32, tag=f"pss{h}",
                                     name=f"pss{b}_{qb}_{cp}_{h}")
                           for h in range(HPC)]
                    for i in range(2):
                        c = 2 * cp + i
                        for h in range(HPC):
                            hb = h * 64
                            nc.tensor.matmul(
                                pss[h][:, i, :],
                                khT[b][hb:hb + 64, c * 128:(c + 1) * 128],
                                qhT[b][hb:hb + 64, q0:q0 + 512],
                                start=True, stop=True,
                                tile_position=(hb, 0))
                    # filler drains BEFORE do_av so streamed vh writes
                    # precede their AV readers in program order
                    while fi < quota(qb * NCP + cp):
                        filler[fi]()
                        fi += 1
                    if pend is not None:
                        do_av(*pend)
                    ets = []
                    for h in range(HPC):
                        e = expp.tile([128, 2, 512], BF16, tag=f"exp{h}")
                        nc.scalar.activation(e[:], pss[h][:], AFT.Exp,
                                             scale=0.125)
                        ets.append(e)
                    pend = (cp, ets)
                do_av(*pend)
                # normalization: 1/sums broadcast, applied on eviction
                for h in range(HPC):
                    ssum = stage.tile([1, 512], F32, tag=f"ssum{h}")
                    nc.vector.tensor_copy(ssum[:], pctx[h][DK:DK + 1, :])
                    si = stage.tile([1, 512], F32, tag=f"sinv{h}")
                    nc.vector.reciprocal_approx_fast(si[:], ssum[:])
                    sbc = stage.tile([64, 512], F32, tag=f"sbc{h}")
                    nc.gpsimd.partition_broadcast(sbc[:], si[:])
                    if h == 0:
                        nc.vector.tensor_mul(
                            ctxT[b][0:64, q0:q0 + 512],
                            pctx[0][0:64, :], sbc[:])
                    else:
                        cs = stage.tile([64, 512], BF16, tag="cstage")
                        nc.vector.tensor_mul(cs[:], pctx[1][0:64, :], sbc[:])
                        nc.sync.dma_start(
                            ctxT[b][64:128, q0:q0 + 512], cs[:])
            for f in filler[fi:]:
                f()

        # ================= pipeline =================
        # DMA issue order tracks consumption: k0 fully + q0-j0 up front
        # (attention starts on them), then v/q rounds, then all of b1.
        order0 = [(0, 0), (0, 1), (1, 0), (0, 2), (0, 3), (2, 0), (1, 1),
                  (2, 1), (1, 2), (2, 2), (1, 3), (2, 3)]
        order1 = [(0, 0), (0, 1), (1, 0), (0, 2), (0, 3), (2, 0), (1, 1),
                  (2, 1), (1, 2), (2, 2), (1, 3), (2, 3)]
        tiles0 = issue_input_dma(
            0, order0, fine={(0, 0), (0, 1), (0, 2), (0, 3), (1, 0)})
        late_consts()
        with tc.tile_pool(name="patt", bufs=1, space="PSUM") as patt:
            # pre-attention: k0 all groups + q0-j0 (feeds qb0), in a
            # double-buffered pool so groups pipeline
            with tc.tile_pool(name="pp0", bufs=2, space="PSUM") as pp0:
                for ti, j in ((0, 0), (0, 1), (0, 2), (0, 3), (1, 0)):
                    proj_group(0, tiles0, pp0, ti, j)
            with tc.tile_pool(name="pp1", bufs=1, space="PSUM") as pp1, \
                 tc.tile_pool(name="ptr1", bufs=1, space="PSUM") as ptr1:

                def vhf(b, tiles, h, j):
                    def f():
                        for c in range(4 * j, 4 * j + 4):
                            vh_unit(b, ptr1, h, c)
                    return f

                # streamed remainder of b0: v/q groups + transposes, with
                # deadlines matching AV chunk consumption (front units 2/cp)
                filler0 = []
                for unit in ((1, 1), (2, 0), "t00", "t10", (2, 1), "t01",
                             "t11", (1, 2), (2, 2), "t02", "t12", (1, 3),
                             (2, 3), "t03", "t13"):
                    if isinstance(unit, tuple):
                        filler0.append(lambda ti=unit[0], j=unit[1]:
                                       proj_group(0, tiles0, pp1, ti, j))
                    else:
                        filler0.append(vhf(0, tiles0, int(unit[1]), int(unit[2])))
                # then all of b1
                tiles1 = issue_input_dma(1, order1)
                for ti, j in order1:
                    filler0.append(lambda ti=ti, j=j:
                                   proj_group(1, tiles1, pp1, ti, j))
                    if ti == 2:
                        for h in range(HPC):
                            filler0.append(vhf(1, tiles1, h, j))
                attention_batch(0, patt, filler0, front=16)

            with tc.tile_pool(name="pout0", bufs=2, space="PSUM") as pout0:
                filler1 = [lambda od=od, qb=qb: outproj_unit(0, pout0, od, qb)
                           for qb in range(NQB) for od in range(8)]
                filler1 += [lambda od=od, qb=qb: outproj_unit(1, pout0, od, qb)
                            for qb in range(NQB - 1) for od in range(8)]
                attention_batch(1, patt, filler1)

        with tc.tile_pool(name="pout1", bufs=4, space="PSUM") as pout1:
            for od in range(8):
                outproj_unit(1, pout1, od, NQB - 1,
                             evict_act=(od % 2 == 1))

    nc.compile()
    return nc


def _get_compiled():
    global _compiled
    if _compiled is None:
        _compiled = _build()
    return _compiled


def _xT(x):
    xf = np.asarray(x, np.float32).reshape(TOK, DM)
    return np.ascontiguousarray(xf.T).astype(ml_dtypes.bfloat16)


def _wshuf(W, cs):
    # [1024, 128] core slice -> [p, c*128+m] so SBUF [128, KC, CW] DMAs clean
    Wc = np.asarray(W, np.float32)[:, cs]
    return np.ascontiguousarray(
        Wc.reshape(KC, 128, CW).transpose(1, 0, 2).reshape(128, KC * CW)
    ).astype(ml_dtypes.bfloat16)


def kernel(q, k, v, Wq, bq, Wk, bk, Wv, bv, Wo, bo):
    global LAST_EXEC_NS
    nc = _get_compiled()

    qTa, kTa, vTa = _xT(q), _xT(k), _xT(v)

    bq, bk, bv = (np.asarray(a, np.float32) for a in (bq, bk, bv))
    Wo = np.asarray(Wo, np.float32)
    bo = np.asarray(bo, np.float32)

    in_maps = []
    for c in range(NCORES):
        cs = slice(c * CW, (c + 1) * CW)
        in_maps.append({
            "qT": qTa, "kT": kTa, "vT": vTa,
            "wq": _wshuf(Wq, cs), "wk": _wshuf(Wk, cs), "wv": _wshuf(Wv, cs),
            "bq": np.ascontiguousarray(bq[cs].reshape(CW, 1)),
            "bk": np.ascontiguousarray(bk[cs].reshape(CW, 1)),
            "bv": np.ascontiguousarray(bv[cs].reshape(CW, 1)),
            "wo": np.ascontiguousarray(Wo[cs, :]).astype(ml_dtypes.bfloat16),
            "bo8": np.ascontiguousarray((bo / NCORES).reshape(8, 128).T),
            "onesd": _ONES,
        })

    kwargs = {}
    if TRACE:
        try:
            import ntff_shim
            ntff_shim.install()
            kwargs["trace"] = True
        except Exception:
            pass

    res = bass_utils.run_bass_kernel_spmd(
        nc, in_maps, core_ids=list(range(NCORES)), **kwargs)
    LAST_EXEC_NS = res.exec_time_ns

    total = res.results[0]["out"].astype(np.float32).copy()
    for c in range(1, NCORES):
        total += res.results[c]["out"]
    return np.ascontiguousarray(total.T).reshape(B, S, DM)


# revision 18
# speedup vs baseline: 1.0200x; 1.0200x over previous
"""8-core SPMD multi-head attention kernel for Trainium2 (Bass/Tile).

Problem: nn.MultiHeadAttention, B=2, S=2048, d_model=1024, 16 heads (dk=64).

Sharding: tensor-parallel over heads — 2 heads per core. Q/K/V projection
weights are column-split per core, out-projection row-split; each core
produces a partial [1024, 4096] output that the host sums.

v3 structure: one monolithic attention stream over (batch, 512-q-block)
with filler work interleaved between chunk-pairs so the PE never idles
long (long PE gaps were measured to reset the clock boost and cost ~35us):
  - proj(b0) + vh(b0) up front with j-interleaved streamed DMA;
  - attention(b0) with proj(b1)+vh(b1) interleaved (PSUM: attention needs
    only 6 banks in the [128, 2, 512] chunk-paired layout, leaving 2);
  - attention(b1) with outproj(b0) interleaved;
  - outproj(b1) tail.
Scores psum is [128, 2, 512] (chunk-paired): exp stays 1024-wide (ACT
cost unchanged) and the pair axis is exactly the fp8 DoubleRow k-subtile
layout, so AV runs e4m3 DoubleRow at half cost. Out-projection in bf16.
"""

import sys

sys.path.insert(0, "/opt/trn_rl_repo")

import numpy as np
import ml_dtypes

import concourse.bass as bass  # noqa: F401
import concourse.mybir as mybir
import concourse.tile as tile
from concourse import bacc
from concourse import bass_utils
from concourse.masks import make_identity

B, S, DM, H, DK = 2, 2048, 1024, 16, 64
TOK = B * S
NCORES = 8
HPC = H // NCORES    # 2 heads per core
CW = HPC * DK        # 128 = per-core qkv width
KC = DM // 128       # 8 contraction chunks
KCH = S // 128       # 16 k-token chunks per batch
NCP = KCH // 2       # 8 chunk-pairs
NQB = S // 512       # 4 q-blocks per batch
F32 = mybir.dt.float32
BF16 = mybir.dt.bfloat16
FP8 = mybir.dt.float8e4
AFT = mybir.ActivationFunctionType
DR = mybir.MatmulPerfMode.DoubleRow

TRACE = False
LAST_EXEC_NS = None

_compiled = None
_ONES = np.ones((128, 64), np.float32).astype(ml_dtypes.bfloat16)


def _build():
    nc = bacc.Bacc("TRN2", target_bir_lowering=False, debug=False,
                   num_devices=NCORES)

    qT = nc.dram_tensor("qT", [DM, TOK], BF16, kind="ExternalInput").ap()
    kT = nc.dram_tensor("kT", [DM, TOK], BF16, kind="ExternalInput").ap()
    vT = nc.dram_tensor("vT", [DM, TOK], BF16, kind="ExternalInput").ap()
    wq = nc.dram_tensor("wq", [128, KC * CW], BF16, kind="ExternalInput").ap()
    wk = nc.dram_tensor("wk", [128, KC * CW], BF16, kind="ExternalInput").ap()
    wv = nc.dram_tensor("wv", [128, KC * CW], BF16, kind="ExternalInput").ap()
    bq = nc.dram_tensor("bq", [128, 1], F32, kind="ExternalInput").ap()
    bk = nc.dram_tensor("bk", [128, 1], F32, kind="ExternalInput").ap()
    bv = nc.dram_tensor("bv", [128, 1], F32, kind="ExternalInput").ap()
    wo = nc.dram_tensor("wo", [CW, DM], BF16, kind="ExternalInput").ap()
    bo8 = nc.dram_tensor("bo8", [128, 8], F32, kind="ExternalInput").ap()
    onesd = nc.dram_tensor("onesd", [128, 64], BF16, kind="ExternalInput").ap()
    out = nc.dram_tensor("out", [DM, TOK], BF16, kind="ExternalOutput").ap()

    def xview(x):
        # [p, c, t]: partition p of chunk c holds feature c*128+p
        return x.rearrange("(c p) t -> p c t", c=KC)

    with tile.TileContext(nc) as tc, \
         tc.tile_pool(name="const", bufs=1) as const, \
         tc.tile_pool(name="xin", bufs=48) as xin, \
         tc.tile_pool(name="expp", bufs=2) as expp, \
         tc.tile_pool(name="stage", bufs=2) as stage, \
         tc.tile_pool(name="outst", bufs=4) as outst:

        # ---------- constants & persistent buffers ----------
        wq_sb = const.tile([128, KC, CW], BF16, tag="wq")
        wk_sb = const.tile([128, KC, CW], BF16, tag="wk")
        wv_sb = const.tile([128, KC, CW], BF16, tag="wv")
        bq_sb = const.tile([128, 1], F32, tag="bq")
        bk_sb = const.tile([128, 1], F32, tag="bk")
        bv_sb = const.tile([128, 1], F32, tag="bv")

        def wpieces(sb, dram):
            dv = dram.rearrange("p (c m) -> p c m", c=KC)
            for i in range(4):
                nc.sync.dma_start(sb[:, 2 * i:2 * i + 2, :],
                                  dv[:, 2 * i:2 * i + 2, :])

        # k weights first: the first projection group only waits on these
        wpieces(wk_sb, wk)
        nc.sync.dma_start(bk_sb[:], bk[:])
        wpieces(wq_sb, wq)
        nc.sync.dma_start(bq_sb[:], bq[:])
        wo_sb = const.tile([CW, DM], BF16, tag="wo")
        bo_sb = const.tile([128, 8], F32, tag="bo")
        ident = const.tile([128, 128], BF16, tag="ident")
        ones_sb = const.tile([128, 64], BF16, tag="ones_sb")

        qhT = [const.tile([128, S], BF16, tag=f"qhT{b}", name=f"qhT{b}")
               for b in range(B)]
        khT = [const.tile([128, S], BF16, tag=f"khT{b}", name=f"khT{b}")
               for b in range(B)]
        vhT = [const.tile([128, S], BF16, tag=f"vhT{b}", name=f"vhT{b}")
               for b in range(B)]
        ctxT = [const.tile([128, S], BF16, tag=f"ctxT{b}", name=f"ctxT{b}")
                for b in range(B)]
        vh = const.tile([128, B, HPC, KCH, DK + 1], BF16, tag="vh")

        PROJ = ((kT, wk_sb, bk_sb, khT), (qT, wq_sb, bq_sb, qhT),
                (vT, wv_sb, bv_sb, vhT))

        def late_consts():
            wpieces(wv_sb, wv)
            nc.sync.dma_start(bv_sb[:], bv[:])
            nc.sync.dma_start(wo_sb[:, 0:512], wo[:, 0:512])
            nc.sync.dma_start(wo_sb[:, 512:1024], wo[:, 512:1024])
            nc.sync.dma_start(bo_sb[:], bo8[:])
            make_identity(nc, ident[:])
            nc.sync.dma_start(ones_sb[:], onesd[:])
            nc.vector.tensor_copy(
                vh[:, :, :, :, DK:DK + 1],
                ones_sb[:].rearrange("p (a b c d) -> p a b c d",
                                     a=B, b=HPC, c=KCH, d=1))

        def issue_input_dma(b, order, fine=()):
            # j-round-interleaved so early chunks of k/q/v all land early;
            # groups in `fine` split per-chunk so transfers fan across queues
            tiles = {}
            for ti, j in order:
                xv = xview(PROJ[ti][0])
                t0 = b * S + j * 512
                for cp in range(4):
                    t = xin.tile([128, 2, 512], BF16, tag="xt")
                    if (ti, j) in fine:
                        for i in range(2):
                            nc.sync.dma_start(
                                t[:, i, :],
                                xv[:, 2 * cp + i, t0:t0 + 512])
                    else:
                        nc.sync.dma_start(
                            t[:], xv[:, 2 * cp:2 * cp + 2, t0:t0 + 512])
                    tiles[(ti, j, cp)] = t
            return tiles

        def proj_group(b, tiles, pp, ti, j, evict_act=False):
            _, w_sb, b_sb, dst = PROJ[ti]
            ps = pp.tile([128, 512], F32, tag="pp", name=f"pp{b}{ti}{j}")
            for c in range(KC):
                nc.tensor.matmul(
                    ps[:], w_sb[:, c, :],
                    tiles[(ti, j, c // 2)][:, c % 2, :],
                    start=(c == 0), stop=(c == KC - 1))
            d = dst[b][:, j * 512:(j + 1) * 512]
            if evict_act:
                nc.scalar.activation(d, ps[:], AFT.Identity, bias=b_sb[:])
            else:
                nc.vector.tensor_scalar_add(d, ps[:], b_sb[:])

        def vh_unit(b, ptr, h, c, evict_act=False):
            pt = ptr.tile([128, 64], BF16, tag="pt")
            hb = h * 64
            nc.tensor.transpose(
                pt[:], vhT[b][hb:hb + 64, c * 128:(c + 1) * 128],
                ident[hb:hb + 64, hb:hb + 64])
            if evict_act:
                nc.scalar.activation(vh[:, b, h, c, 0:DK], pt[:], AFT.Copy)
            else:
                nc.vector.tensor_copy(vh[:, b, h, c, 0:DK], pt[:])

        def outproj_unit(b, pout, od, qb, evict_act=False):
            po = pout.tile([128, 512], F32, tag="po")
            nc.tensor.matmul(
                po[:], wo_sb[:, od * 128:(od + 1) * 128],
                ctxT[b][:, qb * 512:(qb + 1) * 512],
                start=True, stop=True)
            ot = outst.tile([128, 512], BF16, tag="ot")
            if evict_act:
                nc.scalar.activation(ot[:], po[:], AFT.Identity,
                                     bias=bo_sb[:, od:od + 1])
            else:
                nc.vector.tensor_scalar_add(ot[:], po[:], bo_sb[:, od:od + 1])
            nc.sync.dma_start(
                out[od * 128:(od + 1) * 128,
                    b * S + qb * 512:b * S + (qb + 1) * 512],
                ot[:])

        def attention_batch(b, patt, filler, front=0):
            """Attention for batch b over 4 q-blocks of 512, draining
            `filler` between chunk-pairs; the first `front` units drain at
            2/cp (data-streaming deadlines), the rest evenly."""
            fi = 0

            def quota(slot):
                # slot = global cp index 0..31
                got = min(2 * (slot + 1), front)
                rest = len(filler) - front
                if rest > 0:
                    got += (slot + 1) * rest // 32
                return min(got, len(filler))
            for qb in range(NQB):
                q0 = qb * 512
                pctx = [patt.tile([DK + 1, 512], F32, tag=f"pctx{h}",
                                  name=f"pctx{b}_{qb}_{h}")
                        for h in range(HPC)]

                def do_av(cp, ets):
                    for h in range(HPC):
                        for i in range(2):
                            nc.tensor.matmul(
                                pctx[h][:], vh[:, b, h, 2 * cp + i, :],
                                ets[h][:, i, :],
                                start=(cp == 0 and i == 0),
                                stop=(cp == NCP - 1 and i == 1))

                pend = None
                for cp in range(NCP):
                    pss = [patt.tile([128, 2, 512], F32, tag=f"pss{h}",
                                     name=f"pss{b}_{qb}_{cp}_{h}")
                           for h in range(HPC)]
                    for i in range(2):
                        c = 2 * cp + i
                        for h in range(HPC):
                            hb = h * 64
                            nc.tensor.matmul(
                                pss[h][:, i, :],
                                khT[b][hb:hb + 64, c * 128:(c + 1) * 128],
                                qhT[b][hb:hb + 64, q0:q0 + 512],
                                start=True, stop=True,
                                tile_position=(hb, 0))
                    # filler drains BEFORE do_av so streamed vh writes
                    # precede their AV readers in program order
                    while fi < quota(qb * NCP + cp):
                        filler[fi]()
                        fi += 1
                    if pend is not None:
                        do_av(*pend)
                    ets = []
                    for h in range(HPC):
                        e = expp.tile([128, 2, 512], BF16, tag=f"exp{h}")
                        nc.scalar.activation(e[:], pss[h][:], AFT.Exp,
                                             scale=0.125)
                        ets.append(e)
                    pend = (cp, ets)
                do_av(*pend)
                # normalization: 1/sums broadcast, applied on eviction
                for h in range(HPC):
                    ssum = stage.tile([1, 512], F32, tag=f"ssum{h}")
                    nc.vector.tensor_copy(ssum[:], pctx[h][DK:DK + 1, :])
                    si = stage.tile([1, 512], F32, tag=f"sinv{h}")
                    nc.vector.reciprocal_approx_fast(si[:], ssum[:])
                    sbc = stage.tile([64, 512], F32, tag=f"sbc{h}")
                    nc.gpsimd.partition_broadcast(sbc[:], si[:])
                    if h == 0:
                        nc.vector.tensor_mul(
                            ctxT[b][0:64, q0:q0 + 512],
                            pctx[0][0:64, :], sbc[:])
                    else:
                        cs = stage.tile([64, 512], BF16, tag="cstage")
                        nc.vector.tensor_mul(cs[:], pctx[1][0:64, :], sbc[:])
                        nc.sync.dma_start(
                            ctxT[b][64:128, q0:q0 + 512], cs[:])
            for f in filler[fi:]:
                f()

        # ================= pipeline =================
        # DMA issue order tracks consumption: k0 fully + q0-j0 up front
        # (attention starts on them), then v/q rounds, then all of b1.
        order0 = [(0, 0), (0, 1), (1, 0), (0, 2), (0, 3), (2, 0), (1, 1),
                  (2, 1), (1, 2), (2, 2), (1, 3), (2, 3)]
        order1 = [(0, 0), (0, 1), (1, 0), (0, 2), (0, 3), (2, 0), (1, 1),
                  (2, 1), (1, 2), (2, 2), (1, 3), (2, 3)]
        crit = [(0, 0), (0, 1), (0, 2), (0, 3), (1, 0)]
        tiles0 = issue_input_dma(0, crit, fine=set(crit))
        late_consts()
        tiles0.update(issue_input_dma(
            0, [o for o in order0 if o not in crit]))
        with tc.tile_pool(name="patt", bufs=1, space="PSUM") as patt:
            # pre-attention: k0 all groups + q0-j0 (feeds qb0), in a
            # double-buffered pool so groups pipeline
            with tc.tile_pool(name="pp0", bufs=2, space="PSUM") as pp0:
                for ti, j in ((0, 0), (0, 1), (0, 2), (0, 3), (1, 0)):
                    proj_group(0, tiles0, pp0, ti, j)
            with tc.tile_pool(name="pp1", bufs=1, space="PSUM") as pp1, \
                 tc.tile_pool(name="ptr1", bufs=1, space="PSUM") as ptr1:

                def vhf(b, tiles, h, j):
                    def f():
                        for c in range(4 * j, 4 * j + 4):
                            vh_unit(b, ptr1, h, c)
                    return f

                # streamed remainder of b0: v/q groups + transposes, with
                # deadlines matching AV chunk consumption (front units 2/cp)
                filler0 = []
                for unit in ((1, 1), (2, 0), "t00", "t10", (2, 1), "t01",
                             "t11", (1, 2), (2, 2), "t02", "t12", (1, 3),
                             (2, 3), "t03", "t13"):
                    if isinstance(unit, tuple):
                        filler0.append(lambda ti=unit[0], j=unit[1]:
                                       proj_group(0, tiles0, pp1, ti, j))
                    else:
                        filler0.append(vhf(0, tiles0, int(unit[1]), int(unit[2])))
                # then all of b1
                tiles1 = issue_input_dma(1, order1)
                for ti, j in order1:
                    filler0.append(lambda ti=ti, j=j:
                                   proj_group(1, tiles1, pp1, ti, j))
                    if ti == 2:
                        for h in range(HPC):
                            filler0.append(vhf(1, tiles1, h, j))
                attention_batch(0, patt, filler0, front=16)

            with tc.tile_pool(name="pout0", bufs=2, space="PSUM") as pout0:
                filler1 = [lambda od=od, qb=qb: outproj_unit(0, pout0, od, qb)
                           for qb in range(NQB) for od in range(8)]
                filler1 += [lambda od=od, qb=qb: outproj_unit(1, pout0, od, qb)
                            for qb in range(NQB - 1) for od in range(8)]
                attention_batch(1, patt, filler1)

        with tc.tile_pool(name="pout1", bufs=4, space="PSUM") as pout1:
            for od in range(8):
                outproj_unit(1, pout1, od, NQB - 1,
                             evict_act=(od % 2 == 1))

    nc.compile()
    return nc


def _get_compiled():
    global _compiled
    if _compiled is None:
        _compiled = _build()
    return _compiled


def _xT(x):
    xf = np.asarray(x, np.float32).reshape(TOK, DM)
    return np.ascontiguousarray(xf.T).astype(ml_dtypes.bfloat16)


def _wshuf(W, cs):
    # [1024, 128] core slice -> [p, c*128+m] so SBUF [128, KC, CW] DMAs clean
    Wc = np.asarray(W, np.float32)[:, cs]
    return np.ascontiguousarray(
        Wc.reshape(KC, 128, CW).transpose(1, 0, 2).reshape(128, KC * CW)
    ).astype(ml_dtypes.bfloat16)


def kernel(q, k, v, Wq, bq, Wk, bk, Wv, bv, Wo, bo):
    global LAST_EXEC_NS
    nc = _get_compiled()

    qTa, kTa, vTa = _xT(q), _xT(k), _xT(v)

    bq, bk, bv = (np.asarray(a, np.float32) for a in (bq, bk, bv))
    Wo = np.asarray(Wo, np.float32)
    bo = np.asarray(bo, np.float32)

    in_maps = []
    for c in range(NCORES):
        cs = slice(c * CW, (c + 1) * CW)
        in_maps.append({
            "qT": qTa, "kT": kTa, "vT": vTa,
            "wq": _wshuf(Wq, cs), "wk": _wshuf(Wk, cs), "wv": _wshuf(Wv, cs),
            "bq": np.ascontiguousarray(bq[cs].reshape(CW, 1)),
            "bk": np.ascontiguousarray(bk[cs].reshape(CW, 1)),
            "bv": np.ascontiguousarray(bv[cs].reshape(CW, 1)),
            "wo": np.ascontiguousarray(Wo[cs, :]).astype(ml_dtypes.bfloat16),
            "bo8": np.ascontiguousarray((bo / NCORES).reshape(8, 128).T),
            "onesd": _ONES,
        })

    kwargs = {}
    if TRACE:
        try:
            import ntff_shim
            ntff_shim.install()
            kwargs["trace"] = True
        except Exception:
            pass

    res = bass_utils.run_bass_kernel_spmd(
        nc, in_maps, core_ids=list(range(NCORES)), **kwargs)
    LAST_EXEC_NS = res.exec_time_ns

    total = res.results[0]["out"].astype(np.float32).copy()
    for c in range(1, NCORES):
        total += res.results[c]["out"]
    return np.ascontiguousarray(total.T).reshape(B, S, DM)


# revision 19
# speedup vs baseline: 1.0760x; 1.0549x over previous
"""8-core SPMD multi-head attention kernel for Trainium2 (Bass/Tile).

Problem: nn.MultiHeadAttention, B=2, S=2048, d_model=1024, 16 heads (dk=64).

Sharding: tensor-parallel over heads — 2 heads per core. Q/K/V projection
weights are column-split per core, out-projection row-split; each core
produces a partial [1024, 4096] output that the host sums.

v3 structure: one monolithic attention stream over (batch, 512-q-block)
with filler work interleaved between chunk-pairs so the PE never idles
long (long PE gaps were measured to reset the clock boost and cost ~35us):
  - proj(b0) + vh(b0) up front with j-interleaved streamed DMA;
  - attention(b0) with proj(b1)+vh(b1) interleaved (PSUM: attention needs
    only 6 banks in the [128, 2, 512] chunk-paired layout, leaving 2);
  - attention(b1) with outproj(b0) interleaved;
  - outproj(b1) tail.
Scores psum is [128, 2, 512] (chunk-paired): exp stays 1024-wide (ACT
cost unchanged) and the pair axis is exactly the fp8 DoubleRow k-subtile
layout, so AV runs e4m3 DoubleRow at half cost. Out-projection in bf16.
"""

import sys

sys.path.insert(0, "/opt/trn_rl_repo")

import numpy as np
import ml_dtypes

import concourse.bass as bass  # noqa: F401
import concourse.mybir as mybir
import concourse.tile as tile
from concourse import bacc
from concourse import bass_utils
from concourse.masks import make_identity

B, S, DM, H, DK = 2, 2048, 1024, 16, 64
TOK = B * S
NCORES = 8
HPC = H // NCORES    # 2 heads per core
CW = HPC * DK        # 128 = per-core qkv width
KC = DM // 128       # 8 contraction chunks
KCH = S // 128       # 16 k-token chunks per batch
NCP = KCH // 2       # 8 chunk-pairs
NQB = S // 512       # 4 q-blocks per batch
F32 = mybir.dt.float32
BF16 = mybir.dt.bfloat16
FP8 = mybir.dt.float8e4
AFT = mybir.ActivationFunctionType
DR = mybir.MatmulPerfMode.DoubleRow

TRACE = False
LAST_EXEC_NS = None

_compiled = None
_ONES = np.ones((128, 64), np.float32).astype(ml_dtypes.bfloat16)


def _build():
    nc = bacc.Bacc("TRN2", target_bir_lowering=False, debug=False,
                   num_devices=NCORES)

    qT = nc.dram_tensor("qT", [DM, TOK], BF16, kind="ExternalInput").ap()
    kT = nc.dram_tensor("kT", [DM, TOK], BF16, kind="ExternalInput").ap()
    vT = nc.dram_tensor("vT", [DM, TOK], BF16, kind="ExternalInput").ap()
    wq = nc.dram_tensor("wq", [128, KC * CW], BF16, kind="ExternalInput").ap()
    wk = nc.dram_tensor("wk", [128, KC * CW], BF16, kind="ExternalInput").ap()
    wv = nc.dram_tensor("wv", [128, KC * CW], BF16, kind="ExternalInput").ap()
    bq = nc.dram_tensor("bq", [128, 1], F32, kind="ExternalInput").ap()
    bk = nc.dram_tensor("bk", [128, 1], F32, kind="ExternalInput").ap()
    bv = nc.dram_tensor("bv", [128, 1], F32, kind="ExternalInput").ap()
    wo = nc.dram_tensor("wo", [CW, DM], BF16, kind="ExternalInput").ap()
    bo8 = nc.dram_tensor("bo8", [128, 8], F32, kind="ExternalInput").ap()
    onesd = nc.dram_tensor("onesd", [128, 64], BF16, kind="ExternalInput").ap()
    out = nc.dram_tensor("out", [DM, TOK], BF16, kind="ExternalOutput").ap()

    def xview(x):
        # [p, c, t]: partition p of chunk c holds feature c*128+p
        return x.rearrange("(c p) t -> p c t", c=KC)

    with tile.TileContext(nc) as tc, \
         tc.tile_pool(name="const", bufs=1) as const, \
         tc.tile_pool(name="xin", bufs=48) as xin, \
         tc.tile_pool(name="expp", bufs=2) as expp, \
         tc.tile_pool(name="stage", bufs=2) as stage, \
         tc.tile_pool(name="outst", bufs=4) as outst:

        # ---------- constants & persistent buffers ----------
        wq_sb = const.tile([128, KC, CW], BF16, tag="wq")
        wk_sb = const.tile([128, KC, CW], BF16, tag="wk")
        wv_sb = const.tile([128, KC, CW], BF16, tag="wv")
        bq_sb = const.tile([128, 1], F32, tag="bq")
        bk_sb = const.tile([128, 1], F32, tag="bk")
        bv_sb = const.tile([128, 1], F32, tag="bv")

        def wpieces(sb, dram):
            dv = dram.rearrange("p (c m) -> p c m", c=KC)
            for i in range(4):
                nc.sync.dma_start(sb[:, 2 * i:2 * i + 2, :],
                                  dv[:, 2 * i:2 * i + 2, :])

        # k weights first: the first projection group only waits on these
        wpieces(wk_sb, wk)
        nc.sync.dma_start(bk_sb[:], bk[:])
        wpieces(wq_sb, wq)
        nc.sync.dma_start(bq_sb[:], bq[:])
        wpieces(wv_sb, wv)
        nc.sync.dma_start(bv_sb[:], bv[:])
        wo_sb = const.tile([CW, DM], BF16, tag="wo")
        nc.sync.dma_start(wo_sb[:, 0:512], wo[:, 0:512])
        nc.sync.dma_start(wo_sb[:, 512:1024], wo[:, 512:1024])
        bo_sb = const.tile([128, 8], F32, tag="bo")
        nc.sync.dma_start(bo_sb[:], bo8[:])
        ident = const.tile([128, 128], BF16, tag="ident")
        make_identity(nc, ident[:])
        ones_sb = const.tile([128, 64], BF16, tag="ones_sb")
        nc.sync.dma_start(ones_sb[:], onesd[:])

        qhT = [const.tile([128, S], BF16, tag=f"qhT{b}", name=f"qhT{b}")
               for b in range(B)]
        khT = [const.tile([128, S], BF16, tag=f"khT{b}", name=f"khT{b}")
               for b in range(B)]
        vhT = [const.tile([128, S], BF16, tag=f"vhT{b}", name=f"vhT{b}")
               for b in range(B)]
        ctxT = [const.tile([128, S], BF16, tag=f"ctxT{b}", name=f"ctxT{b}")
                for b in range(B)]
        vh = const.tile([128, B, HPC, KCH, DK + 1], BF16, tag="vh")
        nc.vector.tensor_copy(
            vh[:, :, :, :, DK:DK + 1],
            ones_sb[:].rearrange("p (a b c d) -> p a b c d",
                                 a=B, b=HPC, c=KCH, d=1))

        PROJ = ((kT, wk_sb, bk_sb, khT), (qT, wq_sb, bq_sb, qhT),
                (vT, wv_sb, bv_sb, vhT))

        def issue_input_dma(b, order, fine=()):
            tiles = {}
            for ti, j in order:
                xv = xview(PROJ[ti][0])
                t0 = b * S + j * 512
                for cp in range(4):
                    t = xin.tile([128, 2, 512], BF16, tag="xt")
                    if (ti, j) in fine:
                        for i in range(2):
                            nc.sync.dma_start(
                                t[:, i, :], xv[:, 2 * cp + i, t0:t0 + 512])
                    else:
                        nc.sync.dma_start(
                            t[:], xv[:, 2 * cp:2 * cp + 2, t0:t0 + 512])
                    tiles[(ti, j, cp)] = t
            return tiles

        def proj_group(b, tiles, pp, ti, j, evict_act=False):
            _, w_sb, b_sb, dst = PROJ[ti]
            ps = pp.tile([128, 512], F32, tag="pp", name=f"pp{b}{ti}{j}")
            for c in range(KC):
                nc.tensor.matmul(
                    ps[:], w_sb[:, c, :],
                    tiles[(ti, j, c // 2)][:, c % 2, :],
                    start=(c == 0), stop=(c == KC - 1))
            d = dst[b][:, j * 512:(j + 1) * 512]
            if evict_act:
                nc.scalar.activation(d, ps[:], AFT.Identity, bias=b_sb[:])
            else:
                nc.vector.tensor_scalar_add(d, ps[:], b_sb[:])

        def vh_unit(b, ptr, h, c, evict_act=False):
            pt = ptr.tile([128, 64], BF16, tag="pt")
            hb = h * 64
            nc.tensor.transpose(
                pt[:], vhT[b][hb:hb + 64, c * 128:(c + 1) * 128],
                ident[hb:hb + 64, hb:hb + 64])
            if evict_act:
                nc.scalar.activation(vh[:, b, h, c, 0:DK], pt[:], AFT.Copy)
            else:
                nc.vector.tensor_copy(vh[:, b, h, c, 0:DK], pt[:])

        def outproj_unit(b, pout, od, qb, evict_act=False):
            po = pout.tile([128, 512], F32, tag="po")
            nc.tensor.matmul(
                po[:], wo_sb[:, od * 128:(od + 1) * 128],
                ctxT[b][:, qb * 512:(qb + 1) * 512],
                start=True, stop=True)
            ot = outst.tile([128, 512], BF16, tag="ot")
            if evict_act:
                nc.scalar.activation(ot[:], po[:], AFT.Identity,
                                     bias=bo_sb[:, od:od + 1])
            else:
                nc.vector.tensor_scalar_add(ot[:], po[:], bo_sb[:, od:od + 1])
            nc.sync.dma_start(
                out[od * 128:(od + 1) * 128,
                    b * S + qb * 512:b * S + (qb + 1) * 512],
                ot[:])

        def attention_batch(b, patt, filler, front=0):
            """Attention for batch b over 4 q-blocks of 512, draining
            `filler` between chunk-pairs; the first `front` units drain at
            2/cp (data-streaming deadlines), the rest evenly."""
            fi = 0

            def quota(slot):
                # slot = global cp index 0..31
                got = min(2 * (slot + 1), front)
                rest = len(filler) - front
                if rest > 0:
                    got += (slot + 1) * rest // 32
                return min(got, len(filler))
            for qb in range(NQB):
                q0 = qb * 512
                pctx = [patt.tile([DK + 1, 512], F32, tag=f"pctx{h}",
                                  name=f"pctx{b}_{qb}_{h}")
                        for h in range(HPC)]

                def do_av(cp, ets):
                    for h in range(HPC):
                        for i in range(2):
                            nc.tensor.matmul(
                                pctx[h][:], vh[:, b, h, 2 * cp + i, :],
                                ets[h][:, i, :],
                                start=(cp == 0 and i == 0),
                                stop=(cp == NCP - 1 and i == 1))

                pend = None
                for cp in range(NCP):
                    pss = [patt.tile([128, 2, 512], F32, tag=f"pss{h}",
                                     name=f"pss{b}_{qb}_{cp}_{h}")
                           for h in range(HPC)]
                    for i in range(2):
                        c = 2 * cp + i
                        for h in range(HPC):
                            hb = h * 64
                            nc.tensor.matmul(
                                pss[h][:, i, :],
                                khT[b][hb:hb + 64, c * 128:(c + 1) * 128],
                                qhT[b][hb:hb + 64, q0:q0 + 512],
                                start=True, stop=True,
                                tile_position=(hb, 0))
                    # filler drains BEFORE do_av so streamed vh writes
                    # precede their AV readers in program order
                    while fi < quota(qb * NCP + cp):
                        filler[fi]()
                        fi += 1
                    if pend is not None:
                        do_av(*pend)
                    ets = []
                    for h in range(HPC):
                        e = expp.tile([128, 2, 512], BF16, tag=f"exp{h}")
                        nc.scalar.activation(e[:], pss[h][:], AFT.Exp,
                                             scale=0.125)
                        ets.append(e)
                    pend = (cp, ets)
                do_av(*pend)
                # normalization: 1/sums broadcast, applied on eviction
                for h in range(HPC):
                    ssum = stage.tile([1, 512], F32, tag=f"ssum{h}")
                    nc.vector.tensor_copy(ssum[:], pctx[h][DK:DK + 1, :])
                    si = stage.tile([1, 512], F32, tag=f"sinv{h}")
                    nc.vector.reciprocal_approx_fast(si[:], ssum[:])
                    sbc = stage.tile([64, 512], F32, tag=f"sbc{h}")
                    nc.gpsimd.partition_broadcast(sbc[:], si[:])
                    if h == 0:
                        nc.vector.tensor_mul(
                            ctxT[b][0:64, q0:q0 + 512],
                            pctx[0][0:64, :], sbc[:])
                    else:
                        cs = stage.tile([64, 512], BF16, tag="cstage")
                        nc.vector.tensor_mul(cs[:], pctx[1][0:64, :], sbc[:])
                        nc.sync.dma_start(
                            ctxT[b][64:128, q0:q0 + 512], cs[:])
            for f in filler[fi:]:
                f()

        # ================= pipeline =================
        # DMA issue order tracks consumption: k0 fully + q0-j0 up front
        # (attention starts on them), then v/q rounds, then all of b1.
        order0 = [(0, 0), (0, 1), (1, 0), (0, 2), (0, 3), (2, 0), (1, 1),
                  (2, 1), (1, 2), (2, 2), (1, 3), (2, 3)]
        order1 = [(0, 0), (0, 1), (1, 0), (0, 2), (0, 3), (2, 0), (1, 1),
                  (2, 1), (1, 2), (2, 2), (1, 3), (2, 3)]
        tiles0 = issue_input_dma(0, [(0, 0)], fine={(0, 0)})
        tiles0.update(issue_input_dma(0, [o for o in order0 if o != (0, 0)]))
        with tc.tile_pool(name="patt", bufs=1, space="PSUM") as patt:
            # pre-attention: k0 all groups + q0-j0 (feeds qb0), in a
            # double-buffered pool so groups pipeline
            with tc.tile_pool(name="pp0", bufs=2, space="PSUM") as pp0:
                for ti, j in ((0, 0), (0, 1), (0, 2), (0, 3), (1, 0)):
                    proj_group(0, tiles0, pp0, ti, j)
            with tc.tile_pool(name="pp1", bufs=1, space="PSUM") as pp1, \
                 tc.tile_pool(name="ptr1", bufs=1, space="PSUM") as ptr1:

                def vhf(b, tiles, h, j):
                    def f():
                        for c in range(4 * j, 4 * j + 4):
                            vh_unit(b, ptr1, h, c)
                    return f

                # streamed remainder of b0: v/q groups + transposes, with
                # deadlines matching AV chunk consumption (front units 2/cp)
                filler0 = []
                for unit in ((1, 1), (2, 0), "t00", "t10", (2, 1), "t01",
                             "t11", (1, 2), (2, 2), "t02", "t12", (1, 3),
                             (2, 3), "t03", "t13"):
                    if isinstance(unit, tuple):
                        filler0.append(lambda ti=unit[0], j=unit[1]:
                                       proj_group(0, tiles0, pp1, ti, j))
                    else:
                        filler0.append(vhf(0, tiles0, int(unit[1]), int(unit[2])))
                # then all of b1
                tiles1 = issue_input_dma(1, order1)
                for ti, j in order1:
                    filler0.append(lambda ti=ti, j=j:
                                   proj_group(1, tiles1, pp1, ti, j))
                    if ti == 2:
                        for h in range(HPC):
                            filler0.append(vhf(1, tiles1, h, j))
                attention_batch(0, patt, filler0, front=16)

            with tc.tile_pool(name="pout0", bufs=2, space="PSUM") as pout0:
                filler1 = [lambda od=od, qb=qb: outproj_unit(0, pout0, od, qb)
                           for qb in range(NQB) for od in range(8)]
                filler1 += [lambda od=od, qb=qb: outproj_unit(1, pout0, od, qb)
                            for qb in range(NQB - 1) for od in range(8)]
                attention_batch(1, patt, filler1)

        with tc.tile_pool(name="pout1", bufs=4, space="PSUM") as pout1:
            for od in range(8):
                outproj_unit(1, pout1, od, NQB - 1,
                             evict_act=(od % 2 == 1))

    nc.compile()
    return nc


def _get_compiled():
    global _compiled
    if _compiled is None:
        _compiled = _build()
    return _compiled


def _xT(x):
    xf = np.asarray(x, np.float32).reshape(TOK, DM)
    return np.ascontiguousarray(xf.T).astype(ml_dtypes.bfloat16)


def _wshuf(W, cs):
    # [1024, 128] core slice -> [p, c*128+m] so SBUF [128, KC, CW] DMAs clean
    Wc = np.asarray(W, np.float32)[:, cs]
    return np.ascontiguousarray(
        Wc.reshape(KC, 128, CW).transpose(1, 0, 2).reshape(128, KC * CW)
    ).astype(ml_dtypes.bfloat16)


def kernel(q, k, v, Wq, bq, Wk, bk, Wv, bv, Wo, bo):
    global LAST_EXEC_NS
    nc = _get_compiled()

    qTa, kTa, vTa = _xT(q), _xT(k), _xT(v)

    bq, bk, bv = (np.asarray(a, np.float32) for a in (bq, bk, bv))
    Wo = np.asarray(Wo, np.float32)
    bo = np.asarray(bo, np.float32)

    in_maps = []
    for c in range(NCORES):
        cs = slice(c * CW, (c + 1) * CW)
        in_maps.append({
            "qT": qTa, "kT": kTa, "vT": vTa,
            "wq": _wshuf(Wq, cs), "wk": _wshuf(Wk, cs), "wv": _wshuf(Wv, cs),
            "bq": np.ascontiguousarray(bq[cs].reshape(CW, 1)),
            "bk": np.ascontiguousarray(bk[cs].reshape(CW, 1)),
            "bv": np.ascontiguousarray(bv[cs].reshape(CW, 1)),
            "wo": np.ascontiguousarray(Wo[cs, :]).astype(ml_dtypes.bfloat16),
            "bo8": np.ascontiguousarray((bo / NCORES).reshape(8, 128).T),
            "onesd": _ONES,
        })

    kwargs = {}
    if TRACE:
        try:
            import ntff_shim
            ntff_shim.install()
            kwargs["trace"] = True
        except Exception:
            pass

    res = bass_utils.run_bass_kernel_spmd(
        nc, in_maps, core_ids=list(range(NCORES)), **kwargs)
    LAST_EXEC_NS = res.exec_time_ns

    total = res.results[0]["out"].astype(np.float32).copy()
    for c in range(1, NCORES):
        total += res.results[c]["out"]
    return np.ascontiguousarray(total.T).reshape(B, S, DM)
